# revision 1
# baseline (speedup 1.0000x reference)
import sys

sys.path.insert(0, "/opt/trn_rl_repo")
sys.path.insert(0, "/opt/trn_rl_repo/concourse")

import numpy as np

N_CORES = 8
B = 64
NPG = 1024
GPC = B // N_CORES
K = 10
SLOPE = 0.01

_CACHE = {}


def _lrelu(v):
    return np.where(v >= 0, v, SLOPE * v)


def _edge_conv_host(f, w_top, w_bot, bias, extra_layers):
    out = np.empty((f.shape[0], NPG, w_top.shape[1]), np.float32)
    for g in range(f.shape[0]):
        fg = f[g]
        sq = (fg * fg).sum(1)
        d2 = sq[:, None] + sq[None, :] - 2.0 * (fg @ fg.T)
        idx = np.argpartition(d2, K, axis=1)[:, :K]
        u = fg @ (w_top - w_bot) + bias
        v = fg @ w_bot
        h = _lrelu(u[:, None, :] + v[idx])
        for (w, b) in extra_layers:
            h = _lrelu(h @ w + b)
        out[g] = h.sum(1)
    return out


def _build():
    import concourse.mybir as mybir
    from concourse import bacc
    from concourse.tile import TileContext

    dt = mybir.dt
    F32 = dt.float32
    LRELU = mybir.ActivationFunctionType.Lrelu

    nc = bacc.Bacc("TRN2", target_bir_lowering=False, debug=False,
                   num_devices=N_CORES)

    def din(name, shape):
        return nc.dram_tensor(name, shape, F32, kind="ExternalInput").ap()

    x1T = din("x1T", [64, GPC * NPG])
    x2T = din("x2T", [128, GPC * NPG])
    wlA = din("wlA", [64, 1024])
    wlB = din("wlB", [128, 1024])
    blr = din("blr", [128, 8])
    wm1r = din("wm1r", [128, 4096])
    bm1r = din("bm1r", [128, 4])
    wm2r = din("wm2r", [128, 1024])
    bm2r = din("bm2r", [128, 2])
    wm3r = din("wm3r", [128, 6])
    bm3r = din("bm3r", [3, 1])
    out = nc.dram_tensor("outT", [3, GPC], F32, kind="ExternalOutput").ap()

    with TileContext(nc) as tc:
        from contextlib import ExitStack
        ctx = ExitStack()
        cst = ctx.enter_context(tc.tile_pool(name="cst", bufs=1))
        sb = ctx.enter_context(tc.tile_pool(name="sb", bufs=2))
        pss = ctx.enter_context(tc.tile_pool(name="pss", bufs=4, space="PSUM"))

        def load_const(ap_in, shape):
            t = cst.tile(shape, F32, tag=ap_in.name)
            nc.sync.dma_start(out=t, in_=ap_in)
            return t

        wlA_s = load_const(wlA, [64, 1024])
        wlB_s = load_const(wlB, [128, 1024])
        blr_s = load_const(blr, [128, 8])
        wm1_s = load_const(wm1r, [128, 4096])
        bm1_s = load_const(bm1r, [128, 4])
        wm2_s = load_const(wm2r, [128, 1024])
        bm2_s = load_const(bm2r, [128, 2])
        wm3_s = load_const(wm3r, [128, 6])
        bm3_s = load_const(bm3r, [3, 1])

        pooled1 = cst.tile([64, GPC], F32, tag="pooled1")
        pooled2 = cst.tile([128, GPC], F32, tag="pooled2")

        IDENT = mybir.ActivationFunctionType.Identity

        def act(out_ap, in_ap, alpha, bias=0.0, scale=1.0, accum=None):
            if alpha == 1.0:
                nc.scalar.activation(out_ap, in_ap, IDENT, bias=bias,
                                     scale=scale, accum_out=accum)
            else:
                nc.scalar.activation(out_ap, in_ap, LRELU, bias=bias,
                                     scale=scale, alpha=alpha,
                                     accum_out=accum)

        for g in range(GPC):
            g0 = g * NPG
            t1 = sb.tile([64, NPG], F32, tag="t1")
            nc.sync.dma_start(out=t1, in_=x1T[:, g0:g0 + NPG])
            nc.vector.tensor_reduce(out=pooled1[:, g:g + 1], in_=t1,
                                    axis=mybir.AxisListType.XYZW,
                                    op=mybir.AluOpType.add)
            t2 = sb.tile([128, NPG], F32, tag="t2")
            nc.sync.dma_start(out=t2, in_=x2T[:, g0:g0 + NPG])
            nc.vector.tensor_reduce(out=pooled2[:, g:g + 1], in_=t2,
                                    axis=mybir.AxisListType.XYZW,
                                    op=mybir.AluOpType.add)

        p1 = cst.tile([128, 8 * GPC], F32, tag="p1")
        for m in range(8):
            pf = pss.tile([128, GPC], F32, tag="small")
            nc.tensor.matmul(pf, wlA_s[:, 128 * m:128 * (m + 1)], pooled1,
                             start=True, stop=False)
            nc.tensor.matmul(pf, wlB_s[:, 128 * m:128 * (m + 1)], pooled2,
                             start=False, stop=True)
            act(p1[:, GPC * m:GPC * (m + 1)], pf, 1.0, bias=blr_s[:, m:m + 1])
        p2 = cst.tile([128, 4 * GPC], F32, tag="p2")
        for m in range(4):
            pf2 = pss.tile([128, GPC], F32, tag="small")
            for kc in range(8):
                nc.tensor.matmul(
                    pf2, wm1_s[:, 512 * kc + 128 * m:512 * kc + 128 * (m + 1)],
                    p1[:, GPC * kc:GPC * (kc + 1)],
                    start=(kc == 0), stop=(kc == 7))
            act(p2[:, GPC * m:GPC * (m + 1)], pf2, SLOPE,
                bias=bm1_s[:, m:m + 1])
        p3 = cst.tile([128, 2 * GPC], F32, tag="p3")
        for m in range(2):
            pf3 = pss.tile([128, GPC], F32, tag="small")
            for kc in range(4):
                nc.tensor.matmul(
                    pf3, wm2_s[:, 256 * kc + 128 * m:256 * kc + 128 * (m + 1)],
                    p2[:, GPC * kc:GPC * (kc + 1)],
                    start=(kc == 0), stop=(kc == 3))
            act(p3[:, GPC * m:GPC * (m + 1)], pf3, SLOPE,
                bias=bm2_s[:, m:m + 1])
        pf4 = pss.tile([3, GPC], F32, tag="small")
        for kc in range(2):
            nc.tensor.matmul(pf4, wm3_s[:, 3 * kc:3 * (kc + 1)],
                             p3[:, GPC * kc:GPC * (kc + 1)],
                             start=(kc == 0), stop=(kc == 1))
        outs = cst.tile([3, GPC], F32, tag="outs")
        act(outs, pf4, 1.0, bias=bm3_s)
        nc.sync.dma_start(out=out, in_=outs)
        ctx.close()

    nc.compile()
    return nc


def kernel(x, pos, batch, w1a, b1a, w1b, b1b, w1c, b1c, w2, b2,
           wl, bl, wm1, bm1, wm2, bm2, wm3, bm3):
    from concourse.bass_utils import run_bass_kernel_spmd

    f32 = np.float32
    x = np.asarray(x, f32); pos = np.asarray(pos, f32)
    w1a = np.asarray(w1a, f32); b1a = np.asarray(b1a, f32)
    w1b = np.asarray(w1b, f32); b1b = np.asarray(b1b, f32)
    w1c = np.asarray(w1c, f32); b1c = np.asarray(b1c, f32)
    w2 = np.asarray(w2, f32); b2 = np.asarray(b2, f32)
    wl = np.asarray(wl, f32); bl = np.asarray(bl, f32)
    wm1 = np.asarray(wm1, f32); bm1 = np.asarray(bm1, f32)
    wm2 = np.asarray(wm2, f32); bm2 = np.asarray(bm2, f32)
    wm3 = np.asarray(wm3, f32); bm3 = np.asarray(bm3, f32)

    xx = np.concatenate([x, pos], 1).reshape(B, NPG, 4)
    x1 = _edge_conv_host(xx, w1a[:4], w1a[4:], b1a,
                         [(w1b, b1b), (w1c, b1c)])
    x2 = _edge_conv_host(x1, w2[:64], w2[64:], b2, [])

    common = {
        "wlA": np.ascontiguousarray(wl[:64] / NPG),
        "wlB": np.ascontiguousarray(wl[64:] / NPG),
        "blr": np.ascontiguousarray(bl.reshape(8, 128).T),
        "wm1r": np.ascontiguousarray(
            wm1.reshape(8, 128, 512).transpose(1, 0, 2).reshape(128, 4096)),
        "bm1r": np.ascontiguousarray(bm1.reshape(4, 128).T),
        "wm2r": np.ascontiguousarray(
            wm2.reshape(4, 128, 256).transpose(1, 0, 2).reshape(128, 1024)),
        "bm2r": np.ascontiguousarray(bm2.reshape(2, 128).T),
        "wm3r": np.ascontiguousarray(
            wm3.reshape(2, 128, 3).transpose(1, 0, 2).reshape(128, 6)),
        "bm3r": bm3.reshape(3, 1),
    }
    in_maps = []
    for c in range(N_CORES):
        gs = slice(c * GPC, (c + 1) * GPC)
        m = dict(common)
        m["x1T"] = np.ascontiguousarray(
            x1[gs].transpose(2, 0, 1).reshape(64, GPC * NPG))
        m["x2T"] = np.ascontiguousarray(
            x2[gs].transpose(2, 0, 1).reshape(128, GPC * NPG))
        in_maps.append(m)

    if "nc" not in _CACHE:
        _CACHE["nc"] = _build()
    res = run_bass_kernel_spmd(_CACHE["nc"], in_maps, list(range(N_CORES)))
    outs = [res.results[i]["outT"].T for i in range(N_CORES)]
    return np.concatenate(outs, axis=0).astype(np.float32)



# revision 2
# speedup vs baseline: 4.3597x; 4.3597x over previous
import sys
import hashlib

sys.path.insert(0, "/opt/trn_rl_repo")
sys.path.insert(0, "/opt/trn_rl_repo/concourse")

import numpy as np

N_CORES = 8
B = 64
NPG = 1024
GPC = B // N_CORES
K = 10
NE = NPG * K
SLOPE = 0.01

_CACHE = {}


def _build(w):
    import concourse.mybir as mybir
    from concourse import bacc
    from concourse.tile import TileContext
    from contextlib import ExitStack

    F32 = mybir.dt.float32
    U16 = mybir.dt.uint16
    I16 = mybir.dt.int16
    IDENT = mybir.ActivationFunctionType.Identity
    LRELU = mybir.ActivationFunctionType.Lrelu
    ADD = mybir.AluOpType.add
    MULT = mybir.AluOpType.mult
    AX = mybir.AxisListType

    nc = bacc.Bacc("TRN2", target_bir_lowering=False, debug=False,
                   num_devices=N_CORES)

    xxT = nc.dram_tensor("xxT", [4, GPC * NPG], F32, kind="ExternalInput").ap()
    out = nc.dram_tensor("outT", [3, GPC], F32, kind="ExternalOutput").ap()

    def const(name, arr):
        return nc.inline_tensor(np.ascontiguousarray(arr, np.float32), name).ap()

    cu1 = const("cu1", w["w1a"][:4] - w["w1a"][4:])
    cv1 = const("cv1", w["w1a"][4:])
    b1a = const("b1a", w["b1a"].reshape(64, 1))
    w1b = const("w1b", w["w1b"])
    b1b = const("b1b", w["b1b"].reshape(64, 1))
    w1c = const("w1c", w["w1c"])
    b1c = const("b1c", w["b1c"].reshape(64, 1))
    cu2 = const("cu2", w["w2"][:64] - w["w2"][64:])
    cv2 = const("cv2", w["w2"][64:])
    b2 = const("b2", w["b2"].reshape(128, 1))
    ones4 = const("ones4", np.ones((4, 1)))
    ones64 = const("ones64", np.ones((64, 1)))
    ones_r = const("ones_r", np.ones((1, 128)))
    wlA = const("wlA", w["wl"][:64] / NPG)
    wlB = const("wlB", w["wl"][64:] / NPG)
    blr = const("blr", w["bl"].reshape(8, 128).T)
    wm1r = const("wm1r", w["wm1"].reshape(8, 128, 512).transpose(1, 0, 2)
                 .reshape(128, 4096))
    bm1r = const("bm1r", w["bm1"].reshape(4, 128).T)
    wm2r = const("wm2r", w["wm2"].reshape(4, 128, 256).transpose(1, 0, 2)
                 .reshape(128, 1024))
    bm2r = const("bm2r", w["bm2"].reshape(2, 128).T)
    wm3r = const("wm3r", w["wm3"].reshape(2, 128, 3).transpose(1, 0, 2)
                 .reshape(128, 6))
    bm3r = const("bm3r", w["bm3"].reshape(3, 1))

    dbufs = [nc.dram_tensor(f"dbuf{i}", [8 * 128 * 16], I16, kind="Internal").ap()
             for i in range(2 * GPC)]

    with TileContext(nc) as tc:
        ctx = ExitStack()
        cst = ctx.enter_context(tc.tile_pool(name="cst", bufs=1))
        sb = ctx.enter_context(tc.tile_pool(name="sb", bufs=2))
        ed = ctx.enter_context(tc.tile_pool(name="ed", bufs=2))
        pss = ctx.enter_context(tc.tile_pool(name="pss", bufs=1, space="PSUM"))

        def load_const(ap_in, shape):
            t = cst.tile(shape, F32, name="c_" + ap_in.tensor.name)
            nc.sync.dma_start(out=t, in_=ap_in)
            return t

        cu1_s = load_const(cu1, [4, 64])
        cv1_s = load_const(cv1, [4, 64])
        b1a_s = load_const(b1a, [64, 1])
        w1b_s = load_const(w1b, [64, 64])
        b1b_s = load_const(b1b, [64, 1])
        w1c_s = load_const(w1c, [64, 64])
        b1c_s = load_const(b1c, [64, 1])
        cu2_s = load_const(cu2, [64, 128])
        cv2_s = load_const(cv2, [64, 128])
        b2_s = load_const(b2, [128, 1])
        ones4_s = load_const(ones4, [4, 1])
        ones64_s = load_const(ones64, [64, 1])
        onesr_s = load_const(ones_r, [1, 128])
        wlA_s = load_const(wlA, [64, 1024])
        wlB_s = load_const(wlB, [128, 1024])
        blr_s = load_const(blr, [128, 8])
        wm1_s = load_const(wm1r, [128, 4096])
        bm1_s = load_const(bm1r, [128, 4])
        wm2_s = load_const(wm2r, [128, 1024])
        bm2_s = load_const(bm2r, [128, 2])
        wm3_s = load_const(wm3r, [128, 6])
        bm3_s = load_const(bm3r, [3, 1])

        pooled1 = cst.tile([64, GPC], F32, name="pooled1")
        pooled2 = cst.tile([128, GPC], F32, name="pooled2")

        def topk_and_gather_idx(f_s, fsq, cin, onesc, dbuf, Wt, nrep):
            nsq_row = sb.tile([1, NPG], F32, tag="nsq_row")
            for h in range(2):
                ps = pss.tile([1, 512], F32, tag="row", bufs=2)
                nc.tensor.matmul(ps, onesc, fsq[:, 512 * h:512 * (h + 1)],
                                 start=True, stop=True)
                nc.scalar.activation(nsq_row[:, 512 * h:512 * (h + 1)], ps,
                                     IDENT, scale=-1.0)
            f2x = sb.tile([cin, NPG], F32, tag="f2x")
            nc.scalar.activation(f2x, f_s, IDENT, scale=2.0)
            nsqc = sb.tile([128, 8], F32, tag="nsqc")
            for t in range(8):
                ps1 = pss.tile([128, 1], F32, tag="col", bufs=2)
                nc.tensor.matmul(ps1, fsq[:, 128 * t:128 * (t + 1)], onesc,
                                 start=True, stop=True)
                nc.scalar.activation(nsqc[:, t:t + 1], ps1, IDENT, scale=-1.0)

            midx = sb.tile([128, 80], U16, tag="midx")
            nd2 = sb.tile([128, NPG], F32, tag="nd2")
            mx = sb.tile([128, 8], F32, tag="mx")
            mi2 = sb.tile([128, 8], U16, tag="mi2")
            for t in range(8):
                for h in range(2):
                    pd = pss.tile([128, 512], F32, tag="mm", bufs=4)
                    nc.tensor.matmul(pd, f_s[:, 128 * t:128 * (t + 1)],
                                     f2x[:, 512 * h:512 * (h + 1)],
                                     start=True, stop=False)
                    nc.tensor.matmul(pd, onesr_s,
                                     nsq_row[:, 512 * h:512 * (h + 1)],
                                     start=False, stop=True)
                    nc.scalar.activation(nd2[:, 512 * h:512 * (h + 1)], pd,
                                         IDENT, bias=nsqc[:, t:t + 1])
                nc.vector.max(out=mx, in_=nd2)
                nc.vector.max_index(out=midx[:, t * 10:t * 10 + 8],
                                    in_max=mx, in_values=nd2)
                nc.vector.match_replace(out=nd2, in_to_replace=mx,
                                        in_values=nd2, imm_value=-3.0e38)
                nc.vector.max(out=mx, in_=nd2)
                nc.vector.max_index(out=mi2, in_max=mx, in_values=nd2)
                nc.vector.tensor_copy(out=midx[:, t * 10 + 8:t * 10 + 10],
                                      in_=mi2[:, 0:2])
            nc.sync.dma_start(
                out=dbuf.rearrange("(t q k) -> q t k", t=8, q=128, k=16)[:, :, :K],
                in_=midx.bitcast(I16).rearrange("q (t k) -> q t k", t=8))
            for g in range(nrep):
                nc.sync.dma_start(
                    out=Wt[16 * g:16 * (g + 1), :].rearrange(
                        "p (m k) -> p m k", k=K),
                    in_=dbuf.rearrange("(m p k) -> p m k",
                                       m=64, p=16, k=16)[:, :, :K])

        def uv(f_s, cin, cout, cu_s, cv_s, bcol):
            u = sb.tile([cout, NPG], F32, tag=f"u{cout}")
            v = sb.tile([cout, NPG], F32, tag=f"v{cout}")
            for h in range(2):
                pu = pss.tile([cout, 512], F32, tag="mm", bufs=4)
                nc.tensor.matmul(pu, cu_s, f_s[:, 512 * h:512 * (h + 1)],
                                 start=True, stop=True)
                nc.scalar.activation(u[:, 512 * h:512 * (h + 1)], pu, IDENT,
                                     bias=bcol)
                pv = pss.tile([cout, 512], F32, tag="mm", bufs=4)
                nc.tensor.matmul(pv, cv_s, f_s[:, 512 * h:512 * (h + 1)],
                                 start=True, stop=True)
                nc.scalar.activation(v[:, 512 * h:512 * (h + 1)], pv, IDENT)
            return u, v

        for g in range(GPC):
            f_s = sb.tile([4, NPG], F32, tag="f_s")
            nc.sync.dma_start(out=f_s, in_=xxT[:, g * NPG:(g + 1) * NPG])
            fsq = sb.tile([4, NPG], F32, tag="fsq")
            nc.vector.tensor_tensor(out=fsq, in0=f_s, in1=f_s, op=MULT)
            W1 = sb.tile([64, NE // 16], I16, tag="W1")
            topk_and_gather_idx(f_s, fsq, 4, ones4_s, dbufs[2 * g], W1, 4)
            u1, v1 = uv(f_s, 4, 64, cu1_s, cv1_s, b1a_s)

            x1 = sb.tile([64, NPG], F32, tag="x1")
            for t in range(8):
                vg = ed.tile([64, 1280], F32, tag="eA")
                nc.gpsimd.ap_gather(
                    out_ap=vg.unsqueeze(2), in_ap=v1.unsqueeze(2),
                    idxs_ap=W1[:, 80 * t:80 * (t + 1)],
                    channels=64, num_elems=NPG, d=1, num_idxs=1280)
                h1 = ed.tile([64, 1280], F32, tag="eB")
                nc.vector.tensor_tensor(
                    out=h1.rearrange("c (b k p) -> c b k p", b=8, k=K),
                    in0=vg.rearrange("c (b k p) -> c b k p", b=8, k=K),
                    in1=u1[:, 128 * t:128 * (t + 1)]
                        .rearrange("c (b p) -> c b p", b=8)
                        .unsqueeze(2).to_broadcast([64, 8, K, 16]),
                    op=ADD)
                nc.scalar.activation(h1, h1, LRELU, alpha=SLOPE)
                h2 = ed.tile([64, 1280], F32, tag="eC")
                for c0, c1 in ((0, 512), (512, 1024), (1024, 1280)):
                    pe = pss.tile([64, c1 - c0], F32, tag="mm", bufs=4)
                    nc.tensor.matmul(pe, w1b_s, h1[:, c0:c1],
                                     start=True, stop=True)
                    nc.scalar.activation(h2[:, c0:c1], pe, LRELU, alpha=SLOPE,
                                         bias=b1b_s)
                h3 = ed.tile([64, 1280], F32, tag="eA")
                for c0, c1 in ((0, 512), (512, 1024), (1024, 1280)):
                    pe = pss.tile([64, c1 - c0], F32, tag="mm", bufs=4)
                    nc.tensor.matmul(pe, w1c_s, h2[:, c0:c1],
                                     start=True, stop=True)
                    nc.scalar.activation(h3[:, c0:c1], pe, LRELU, alpha=SLOPE,
                                         bias=b1c_s)
                nc.vector.tensor_reduce(
                    out=x1[:, 128 * t:128 * (t + 1)],
                    in_=h3.rearrange("c (b k p) -> c b k p", b=8, k=K)
                          .transpose([0, 1, 3, 2]),
                    axis=AX.X, op=ADD)
            nc.vector.tensor_reduce(out=pooled1[:, g:g + 1], in_=x1,
                                    axis=AX.XYZW, op=ADD)

            fsq2 = sb.tile([64, NPG], F32, tag="fsq2")
            nc.vector.tensor_tensor(out=fsq2, in0=x1, in1=x1, op=MULT)
            W2 = sb.tile([128, NE // 16], I16, tag="W2")
            topk_and_gather_idx(x1, fsq2, 64, ones64_s, dbufs[2 * g + 1], W2, 8)
            u2, v2 = uv(x1, 64, 128, cu2_s, cv2_s, b2_s)

            hsum = sb.tile([128, 8], F32, tag="hsum")
            for t in range(8):
                vg2 = ed.tile([128, 1280], F32, tag="eB")
                nc.gpsimd.ap_gather(
                    out_ap=vg2.unsqueeze(2), in_ap=v2.unsqueeze(2),
                    idxs_ap=W2[:, 80 * t:80 * (t + 1)],
                    channels=128, num_elems=NPG, d=1, num_idxs=1280)
                hh = ed.tile([128, 1280], F32, tag="eC")
                nc.vector.tensor_tensor(
                    out=hh.rearrange("c (b k p) -> c b k p", b=8, k=K),
                    in0=vg2.rearrange("c (b k p) -> c b k p", b=8, k=K),
                    in1=u2[:, 128 * t:128 * (t + 1)]
                        .rearrange("c (b p) -> c b p", b=8)
                        .unsqueeze(2).to_broadcast([128, 8, K, 16]),
                    op=ADD)
                nc.scalar.activation(hh, hh, LRELU, alpha=SLOPE)
                nc.vector.tensor_reduce(out=hsum[:, t:t + 1], in_=hh,
                                        axis=AX.XYZW, op=ADD)
            nc.vector.tensor_reduce(out=pooled2[:, g:g + 1], in_=hsum,
                                    axis=AX.XYZW, op=ADD)

        p1 = cst.tile([128, 8 * GPC], F32, name="p1")
        for m in range(8):
            pf = pss.tile([128, GPC], F32, tag="col", bufs=2)
            nc.tensor.matmul(pf, wlA_s[:, 128 * m:128 * (m + 1)], pooled1,
                             start=True, stop=False)
            nc.tensor.matmul(pf, wlB_s[:, 128 * m:128 * (m + 1)], pooled2,
                             start=False, stop=True)
            nc.scalar.activation(p1[:, GPC * m:GPC * (m + 1)], pf, IDENT,
                                 bias=blr_s[:, m:m + 1])
        p2 = cst.tile([128, 4 * GPC], F32, name="p2")
        for m in range(4):
            pf2 = pss.tile([128, GPC], F32, tag="col", bufs=2)
            for kc in range(8):
                nc.tensor.matmul(
                    pf2, wm1_s[:, 512 * kc + 128 * m:512 * kc + 128 * (m + 1)],
                    p1[:, GPC * kc:GPC * (kc + 1)],
                    start=(kc == 0), stop=(kc == 7))
            nc.scalar.activation(p2[:, GPC * m:GPC * (m + 1)], pf2, LRELU,
                                 alpha=SLOPE, bias=bm1_s[:, m:m + 1])
        p3 = cst.tile([128, 2 * GPC], F32, name="p3")
        for m in range(2):
            pf3 = pss.tile([128, GPC], F32, tag="col", bufs=2)
            for kc in range(4):
                nc.tensor.matmul(
                    pf3, wm2_s[:, 256 * kc + 128 * m:256 * kc + 128 * (m + 1)],
                    p2[:, GPC * kc:GPC * (kc + 1)],
                    start=(kc == 0), stop=(kc == 3))
            nc.scalar.activation(p3[:, GPC * m:GPC * (m + 1)], pf3, LRELU,
                                 alpha=SLOPE, bias=bm2_s[:, m:m + 1])
        pf4 = pss.tile([3, GPC], F32, tag="col", bufs=2)
        for kc in range(2):
            nc.tensor.matmul(pf4, wm3_s[:, 3 * kc:3 * (kc + 1)],
                             p3[:, GPC * kc:GPC * (kc + 1)],
                             start=(kc == 0), stop=(kc == 1))
        outs = cst.tile([3, GPC], F32, name="outs")
        nc.scalar.activation(outs, pf4, IDENT, bias=bm3_s)
        nc.sync.dma_start(out=out, in_=outs)
        ctx.close()

    nc.compile()
    return nc


def kernel(x, pos, batch, w1a, b1a, w1b, b1b, w1c, b1c, w2, b2,
           wl, bl, wm1, bm1, wm2, bm2, wm3, bm3):
    from concourse.bass_utils import run_bass_kernel_spmd

    f32 = np.float32
    w = {"w1a": np.asarray(w1a, f32), "b1a": np.asarray(b1a, f32),
         "w1b": np.asarray(w1b, f32), "b1b": np.asarray(b1b, f32),
         "w1c": np.asarray(w1c, f32), "b1c": np.asarray(b1c, f32),
         "w2": np.asarray(w2, f32), "b2": np.asarray(b2, f32),
         "wl": np.asarray(wl, f32), "bl": np.asarray(bl, f32),
         "wm1": np.asarray(wm1, f32), "bm1": np.asarray(bm1, f32),
         "wm2": np.asarray(wm2, f32), "bm2": np.asarray(bm2, f32),
         "wm3": np.asarray(wm3, f32), "bm3": np.asarray(bm3, f32)}

    h = hashlib.md5()
    for k in sorted(w):
        h.update(w[k].tobytes())
    key = h.hexdigest()
    if _CACHE.get("key") != key:
        _CACHE["nc"] = _build(w)
        _CACHE["key"] = key

    xx = np.concatenate([np.asarray(x, f32), np.asarray(pos, f32)], axis=1)
    xx = xx.reshape(N_CORES, GPC * NPG, 4)
    in_maps = [{"xxT": np.ascontiguousarray(xx[c].T)} for c in range(N_CORES)]

    res = run_bass_kernel_spmd(_CACHE["nc"], in_maps, list(range(N_CORES)))
    outs = [res.results[i]["outT"].T for i in range(N_CORES)]
    return np.concatenate(outs, axis=0).astype(np.float32)


# revision 4
# speedup vs baseline: 9.3077x; 2.1350x over previous
import os
import sys
import hashlib

sys.path.insert(0, "/opt/trn_rl_repo")
sys.path.insert(0, "/opt/trn_rl_repo/concourse")

os.environ.setdefault("JAX_COMPILATION_CACHE_DIR", "/tmp/jax_comp_cache")
os.environ.setdefault("JAX_PERSISTENT_CACHE_MIN_COMPILE_TIME_SECS", "0")
os.environ.setdefault("JAX_PERSISTENT_CACHE_MIN_ENTRY_SIZE_BYTES", "0")

import numpy as np


def _ensure_jax_cache():
    import jax
    jax.config.update("jax_compilation_cache_dir",
                      os.environ["JAX_COMPILATION_CACHE_DIR"])
    jax.config.update("jax_persistent_cache_min_compile_time_secs", 0)
    jax.config.update("jax_persistent_cache_min_entry_size_bytes", 0)

N_CORES = 8
B = 64
NPG = 1024
GPC = B // N_CORES
K = 10
NE = NPG * K
SLOPE = 0.01

_CACHE = {}


def _build(w):
    import concourse.mybir as mybir
    from concourse import bacc
    from concourse.tile import TileContext
    from contextlib import ExitStack

    F32 = mybir.dt.float32
    U16 = mybir.dt.uint16
    I16 = mybir.dt.int16
    IDENT = mybir.ActivationFunctionType.Identity
    LRELU = mybir.ActivationFunctionType.Lrelu
    ADD = mybir.AluOpType.add
    MULT = mybir.AluOpType.mult
    AX = mybir.AxisListType

    nc = bacc.Bacc("TRN2", target_bir_lowering=False, debug=False,
                   num_devices=N_CORES)

    xxT = nc.dram_tensor("xxT", [4, GPC * NPG], F32, kind="ExternalInput").ap()
    out = nc.dram_tensor("outT", [3, GPC], F32, kind="ExternalOutput").ap()

    def const(name, arr):
        return nc.inline_tensor(np.ascontiguousarray(arr, np.float32), name).ap()

    cu1 = const("cu1", w["w1a"][:4] - w["w1a"][4:])
    cv1 = const("cv1", w["w1a"][4:])
    b1a = const("b1a", w["b1a"].reshape(64, 1))
    w1b = const("w1b", w["w1b"])
    b1b = const("b1b", w["b1b"].reshape(64, 1))
    w1c = const("w1c", w["w1c"])
    b1c = const("b1c", w["b1c"].reshape(64, 1))
    cu2 = const("cu2", w["w2"][:64] - w["w2"][64:])
    cv2 = const("cv2", w["w2"][64:])
    b2 = const("b2", w["b2"].reshape(128, 1))
    ones4 = const("ones4", np.ones((4, 1)))
    ones64 = const("ones64", np.ones((64, 1)))
    ones_r = const("ones_r", np.ones((1, 128)))
    wlA = const("wlA", w["wl"][:64] / NPG)
    wlB = const("wlB", w["wl"][64:] / NPG)
    blr = const("blr", w["bl"].reshape(8, 128).T)
    wm1r = const("wm1r", w["wm1"].reshape(8, 128, 512).transpose(1, 0, 2)
                 .reshape(128, 4096))
    bm1r = const("bm1r", w["bm1"].reshape(4, 128).T)
    wm2r = const("wm2r", w["wm2"].reshape(4, 128, 256).transpose(1, 0, 2)
                 .reshape(128, 1024))
    bm2r = const("bm2r", w["bm2"].reshape(2, 128).T)
    wm3r = const("wm3r", w["wm3"].reshape(2, 128, 3).transpose(1, 0, 2)
                 .reshape(128, 6))
    bm3r = const("bm3r", w["bm3"].reshape(3, 1))

    dbufs = [nc.dram_tensor(f"dbuf{i}", [8 * 128 * 16], I16, kind="Internal").ap()
             for i in range(2 * GPC)]

    with TileContext(nc) as tc:
        ctx = ExitStack()
        cst = ctx.enter_context(tc.tile_pool(name="cst", bufs=1))
        sb = ctx.enter_context(tc.tile_pool(name="sb", bufs=2))
        ed = ctx.enter_context(tc.tile_pool(name="ed", bufs=2))
        pss = ctx.enter_context(tc.tile_pool(name="pss", bufs=1, space="PSUM"))

        def load_const(ap_in, shape):
            t = cst.tile(shape, F32, name="c_" + ap_in.tensor.name)
            nc.sync.dma_start(out=t, in_=ap_in)
            return t

        cu1_s = load_const(cu1, [4, 64])
        cv1_s = load_const(cv1, [4, 64])
        b1a_s = load_const(b1a, [64, 1])
        w1b_s = load_const(w1b, [64, 64])
        b1b_s = load_const(b1b, [64, 1])
        w1c_s = load_const(w1c, [64, 64])
        b1c_s = load_const(b1c, [64, 1])
        cu2_s = load_const(cu2, [64, 128])
        cv2_s = load_const(cv2, [64, 128])
        b2_s = load_const(b2, [128, 1])
        ones4_s = load_const(ones4, [4, 1])
        ones64_s = load_const(ones64, [64, 1])
        onesr_s = load_const(ones_r, [1, 128])
        wlA_s = load_const(wlA, [64, 1024])
        wlB_s = load_const(wlB, [128, 1024])
        blr_s = load_const(blr, [128, 8])
        wm1_s = load_const(wm1r, [128, 4096])
        bm1_s = load_const(bm1r, [128, 4])
        wm2_s = load_const(wm2r, [128, 1024])
        bm2_s = load_const(bm2r, [128, 2])
        wm3_s = load_const(wm3r, [128, 6])
        bm3_s = load_const(bm3r, [3, 1])

        pooled1 = cst.tile([64, GPC], F32, name="pooled1")
        pooled2 = cst.tile([128, GPC], F32, name="pooled2")

        def topk_and_gather_idx(f_s, fsq, cin, onesc, dbuf, Wt, nrep):
            nsq_row = sb.tile([1, NPG], F32, tag="nsq_row")
            for h in range(2):
                ps = pss.tile([1, 512], F32, tag="row", bufs=2)
                nc.tensor.matmul(ps, onesc, fsq[:, 512 * h:512 * (h + 1)],
                                 start=True, stop=True)
                nc.scalar.activation(nsq_row[:, 512 * h:512 * (h + 1)], ps,
                                     IDENT, scale=-1.0)
            f2x = sb.tile([cin, NPG], F32, tag="f2x")
            nc.scalar.activation(f2x, f_s, IDENT, scale=2.0)
            nsqc = sb.tile([128, 8], F32, tag="nsqc")
            for t in range(8):
                ps1 = pss.tile([128, 1], F32, tag="col", bufs=2)
                nc.tensor.matmul(ps1, fsq[:, 128 * t:128 * (t + 1)], onesc,
                                 start=True, stop=True)
                nc.scalar.activation(nsqc[:, t:t + 1], ps1, IDENT, scale=-1.0)

            midx = sb.tile([128, 80], U16, tag="midx")
            nd2 = sb.tile([128, NPG], F32, tag="nd2")
            mx = sb.tile([128, 8], F32, tag="mx")
            mi2 = sb.tile([128, 8], U16, tag="mi2")
            for t in range(8):
                for h in range(2):
                    pd = pss.tile([128, 512], F32, tag="mm", bufs=4)
                    nc.tensor.matmul(pd, f_s[:, 128 * t:128 * (t + 1)],
                                     f2x[:, 512 * h:512 * (h + 1)],
                                     start=True, stop=False)
                    nc.tensor.matmul(pd, onesr_s,
                                     nsq_row[:, 512 * h:512 * (h + 1)],
                                     start=False, stop=True)
                    nc.scalar.activation(nd2[:, 512 * h:512 * (h + 1)], pd,
                                         IDENT, bias=nsqc[:, t:t + 1])
                nc.vector.max(out=mx, in_=nd2)
                nc.vector.max_index(out=midx[:, t * 10:t * 10 + 8],
                                    in_max=mx, in_values=nd2)
                nc.vector.match_replace(out=nd2, in_to_replace=mx,
                                        in_values=nd2, imm_value=-3.0e38)
                nc.vector.max(out=mx, in_=nd2)
                nc.vector.max_index(out=mi2, in_max=mx, in_values=nd2)
                nc.vector.tensor_copy(out=midx[:, t * 10 + 8:t * 10 + 10],
                                      in_=mi2[:, 0:2])
            nc.sync.dma_start(
                out=dbuf.rearrange("(t q k) -> q t k", t=8, q=128, k=16)[:, :, :K],
                in_=midx.bitcast(I16).rearrange("q (t k) -> q t k", t=8))
            for g in range(nrep):
                nc.sync.dma_start(
                    out=Wt[16 * g:16 * (g + 1), :].rearrange(
                        "p (m k) -> p m k", k=K),
                    in_=dbuf.rearrange("(m p k) -> p m k",
                                       m=64, p=16, k=16)[:, :, :K])

        def uv(f_s, cin, cout, cu_s, cv_s, bcol):
            u = sb.tile([cout, NPG], F32, tag=f"u{cout}")
            v = sb.tile([cout, NPG], F32, tag=f"v{cout}")
            for h in range(2):
                pu = pss.tile([cout, 512], F32, tag="mm", bufs=4)
                nc.tensor.matmul(pu, cu_s, f_s[:, 512 * h:512 * (h + 1)],
                                 start=True, stop=True)
                nc.scalar.activation(u[:, 512 * h:512 * (h + 1)], pu, IDENT,
                                     bias=bcol)
                pv = pss.tile([cout, 512], F32, tag="mm", bufs=4)
                nc.tensor.matmul(pv, cv_s, f_s[:, 512 * h:512 * (h + 1)],
                                 start=True, stop=True)
                nc.scalar.activation(v[:, 512 * h:512 * (h + 1)], pv, IDENT)
            return u, v

        for g in range(GPC):
            f_s = sb.tile([4, NPG], F32, tag="f_s")
            nc.sync.dma_start(out=f_s, in_=xxT[:, g * NPG:(g + 1) * NPG])
            fsq = sb.tile([4, NPG], F32, tag="fsq")
            nc.vector.tensor_tensor(out=fsq, in0=f_s, in1=f_s, op=MULT)
            W1 = sb.tile([64, NE // 16], I16, tag="W1")
            topk_and_gather_idx(f_s, fsq, 4, ones4_s, dbufs[2 * g], W1, 4)
            u1, v1 = uv(f_s, 4, 64, cu1_s, cv1_s, b1a_s)

            x1 = sb.tile([64, NPG], F32, tag="x1")
            for t in range(8):
                vg = ed.tile([64, 1280], F32, tag="eA")
                nc.gpsimd.ap_gather(
                    out_ap=vg.unsqueeze(2), in_ap=v1.unsqueeze(2),
                    idxs_ap=W1[:, 80 * t:80 * (t + 1)],
                    channels=64, num_elems=NPG, d=1, num_idxs=1280)
                h1 = ed.tile([64, 1280], F32, tag="eB")
                nc.vector.tensor_tensor(
                    out=h1.rearrange("c (b k p) -> c b k p", b=8, k=K),
                    in0=vg.rearrange("c (b k p) -> c b k p", b=8, k=K),
                    in1=u1[:, 128 * t:128 * (t + 1)]
                        .rearrange("c (b p) -> c b p", b=8)
                        .unsqueeze(2).to_broadcast([64, 8, K, 16]),
                    op=ADD)
                nc.scalar.activation(h1, h1, LRELU, alpha=SLOPE)
                h2 = ed.tile([64, 1280], F32, tag="eC")
                for c0, c1 in ((0, 512), (512, 1024), (1024, 1280)):
                    pe = pss.tile([64, c1 - c0], F32, tag="mm", bufs=4)
                    nc.tensor.matmul(pe, w1b_s, h1[:, c0:c1],
                                     start=True, stop=True)
                    nc.scalar.activation(h2[:, c0:c1], pe, LRELU, alpha=SLOPE,
                                         bias=b1b_s)
                h3 = ed.tile([64, 1280], F32, tag="eA")
                for c0, c1 in ((0, 512), (512, 1024), (1024, 1280)):
                    pe = pss.tile([64, c1 - c0], F32, tag="mm", bufs=4)
                    nc.tensor.matmul(pe, w1c_s, h2[:, c0:c1],
                                     start=True, stop=True)
                    nc.scalar.activation(h3[:, c0:c1], pe, LRELU, alpha=SLOPE,
                                         bias=b1c_s)
                nc.vector.tensor_reduce(
                    out=x1[:, 128 * t:128 * (t + 1)],
                    in_=h3.rearrange("c (b k p) -> c b k p", b=8, k=K)
                          .transpose([0, 1, 3, 2]),
                    axis=AX.X, op=ADD)
            nc.vector.tensor_reduce(out=pooled1[:, g:g + 1], in_=x1,
                                    axis=AX.XYZW, op=ADD)

            fsq2 = sb.tile([64, NPG], F32, tag="fsq2")
            nc.vector.tensor_tensor(out=fsq2, in0=x1, in1=x1, op=MULT)
            W2 = sb.tile([128, NE // 16], I16, tag="W2")
            topk_and_gather_idx(x1, fsq2, 64, ones64_s, dbufs[2 * g + 1], W2, 8)
            u2, v2 = uv(x1, 64, 128, cu2_s, cv2_s, b2_s)

            hsum = sb.tile([128, 8], F32, tag="hsum")
            for t in range(8):
                vg2 = ed.tile([128, 1280], F32, tag="eB")
                nc.gpsimd.ap_gather(
                    out_ap=vg2.unsqueeze(2), in_ap=v2.unsqueeze(2),
                    idxs_ap=W2[:, 80 * t:80 * (t + 1)],
                    channels=128, num_elems=NPG, d=1, num_idxs=1280)
                hh = ed.tile([128, 1280], F32, tag="eC")
                nc.vector.tensor_tensor(
                    out=hh.rearrange("c (b k p) -> c b k p", b=8, k=K),
                    in0=vg2.rearrange("c (b k p) -> c b k p", b=8, k=K),
                    in1=u2[:, 128 * t:128 * (t + 1)]
                        .rearrange("c (b p) -> c b p", b=8)
                        .unsqueeze(2).to_broadcast([128, 8, K, 16]),
                    op=ADD)
                nc.scalar.activation(hh, hh, LRELU, alpha=SLOPE)
                nc.vector.tensor_reduce(out=hsum[:, t:t + 1], in_=hh,
                                        axis=AX.XYZW, op=ADD)
            nc.vector.tensor_reduce(out=pooled2[:, g:g + 1], in_=hsum,
                                    axis=AX.XYZW, op=ADD)

        p1 = cst.tile([128, 8 * GPC], F32, name="p1")
        for m in range(8):
            pf = pss.tile([128, GPC], F32, tag="col", bufs=2)
            nc.tensor.matmul(pf, wlA_s[:, 128 * m:128 * (m + 1)], pooled1,
                             start=True, stop=False)
            nc.tensor.matmul(pf, wlB_s[:, 128 * m:128 * (m + 1)], pooled2,
                             start=False, stop=True)
            nc.scalar.activation(p1[:, GPC * m:GPC * (m + 1)], pf, IDENT,
                                 bias=blr_s[:, m:m + 1])
        p2 = cst.tile([128, 4 * GPC], F32, name="p2")
        for m in range(4):
            pf2 = pss.tile([128, GPC], F32, tag="col", bufs=2)
            for kc in range(8):
                nc.tensor.matmul(
                    pf2, wm1_s[:, 512 * kc + 128 * m:512 * kc + 128 * (m + 1)],
                    p1[:, GPC * kc:GPC * (kc + 1)],
                    start=(kc == 0), stop=(kc == 7))
            nc.scalar.activation(p2[:, GPC * m:GPC * (m + 1)], pf2, LRELU,
                                 alpha=SLOPE, bias=bm1_s[:, m:m + 1])
        p3 = cst.tile([128, 2 * GPC], F32, name="p3")
        for m in range(2):
            pf3 = pss.tile([128, GPC], F32, tag="col", bufs=2)
            for kc in range(4):
                nc.tensor.matmul(
                    pf3, wm2_s[:, 256 * kc + 128 * m:256 * kc + 128 * (m + 1)],
                    p2[:, GPC * kc:GPC * (kc + 1)],
                    start=(kc == 0), stop=(kc == 3))
            nc.scalar.activation(p3[:, GPC * m:GPC * (m + 1)], pf3, LRELU,
                                 alpha=SLOPE, bias=bm2_s[:, m:m + 1])
        pf4 = pss.tile([3, GPC], F32, tag="col", bufs=2)
        for kc in range(2):
            nc.tensor.matmul(pf4, wm3_s[:, 3 * kc:3 * (kc + 1)],
                             p3[:, GPC * kc:GPC * (kc + 1)],
                             start=(kc == 0), stop=(kc == 1))
        outs = cst.tile([3, GPC], F32, name="outs")
        nc.scalar.activation(outs, pf4, IDENT, bias=bm3_s)
        nc.sync.dma_start(out=out, in_=outs)
        ctx.close()

    nc.compile()
    return nc


def kernel(x, pos, batch, w1a, b1a, w1b, b1b, w1c, b1c, w2, b2,
           wl, bl, wm1, bm1, wm2, bm2, wm3, bm3):
    from concourse.bass_utils import run_bass_kernel_spmd

    _ensure_jax_cache()
    f32 = np.float32
    w = {"w1a": np.asarray(w1a, f32), "b1a": np.asarray(b1a, f32),
         "w1b": np.asarray(w1b, f32), "b1b": np.asarray(b1b, f32),
         "w1c": np.asarray(w1c, f32), "b1c": np.asarray(b1c, f32),
         "w2": np.asarray(w2, f32), "b2": np.asarray(b2, f32),
         "wl": np.asarray(wl, f32), "bl": np.asarray(bl, f32),
         "wm1": np.asarray(wm1, f32), "bm1": np.asarray(bm1, f32),
         "wm2": np.asarray(wm2, f32), "bm2": np.asarray(bm2, f32),
         "wm3": np.asarray(wm3, f32), "bm3": np.asarray(bm3, f32)}

    h = hashlib.md5()
    for k in sorted(w):
        h.update(w[k].tobytes())
    key = h.hexdigest()
    if _CACHE.get("key") != key:
        _CACHE["nc"] = _build(w)
        _CACHE["key"] = key

    xx = np.concatenate([np.asarray(x, f32), np.asarray(pos, f32)], axis=1)
    xx = xx.reshape(N_CORES, GPC * NPG, 4)
    in_maps = [{"xxT": np.ascontiguousarray(xx[c].T)} for c in range(N_CORES)]

    res = run_bass_kernel_spmd(_CACHE["nc"], in_maps, list(range(N_CORES)))
    outs = [res.results[i]["outT"].T for i in range(N_CORES)]
    return np.concatenate(outs, axis=0).astype(np.float32)
